# revision 1
# baseline (speedup 1.0000x reference)
"""Cross-attention kernel for 8 Trainium2 NeuronCores.

Contract: kernel(**inputs) takes FULL unsharded numpy inputs
(x [4,2048,1024], context [4,2048,1024], Wq [1024,1024], Wkv [1024,2048])
and returns the full output [4, 2048, 1024] (float32).

Sharding (hardcoded): core = b * 2 + hg handles batch b (0..3) and head
group hg (0..1) = heads hg*8 .. hg*8+7 (16 heads total, d=64). Data +
tensor parallel: no cross-core communication needed (softmax is per-row).

Per-core dataflow (all fp32):
  cT = context[b].T              (PE transpose via identity)
  KT = Wk_slice.T @ cT           [512 c, 2048 j]
  V  = cT.T @ Wv_slice           [2048 j, 8 h, 65]  (col 64 = 1.0)
  xT = x[b].T
  QT = Wq_slice.T @ xT           [512 c, 2048 i]
  per (head h, i-macro of 1024):
    for j-chunk of 128:
      S^T  = K_h^T' Q_h^T        [128 j, 1024 i]  PSUM   (K=64 matmul)
      P^T  = exp(S^T / 8)        ACT, PSUM -> SBUF (no max-sub needed:
                                  scores ~ N(0,1), fp32 exp is safe)
      AT  += [V_h|1].T @ P^T     [65, 1024 i]  PSUM accumulate over j
    AT -> SBUF -> PE-transpose 128-col blocks -> [128 i, 65] PSUM
    out_sb[:, h*64:+64] = AT_t[:, :64] * recip(AT_t[:, 64])   (DVE)
  DMA out_sb -> out[2048, 512] DRAM (host scatters into full output)
"""

import sys

if "/opt/trn_rl_repo" not in sys.path:
    sys.path.insert(0, "/opt/trn_rl_repo")

from contextlib import ExitStack

import numpy as np

import concourse.bass as bass  # noqa: F401  (registers AP machinery)
import concourse.mybir as mybir
from concourse import bacc
from concourse.bass_utils import run_bass_kernel_spmd
from concourse.masks import make_identity
from concourse.tile import TileContext

FP = mybir.dt.float32
P = 128
SEQ = 2048
DIM = 1024
CC = 512  # per-core channel cols (8 heads x 64)
NH = 8  # heads per core
DH = 64  # head dim
NI = SEQ // P  # 16 seq chunks
NK = DIM // P  # 8 contraction chunks
IM = 1024  # i-macro width for attention
NIM = SEQ // IM  # 2
SCALE = DH ** -0.5

EXP = mybir.ActivationFunctionType.Exp

_NC = None


def _build_body(nc, tc, x_d, c_d, wq_d, wk_d, wv_d, out_d):
    with ExitStack() as ctx:
        const = ctx.enter_context(tc.tile_pool(name="const", bufs=1))
        ident = const.tile([P, P], FP, name="ident")
        make_identity(nc, ident)

        # persistent across phases
        ktp = ctx.enter_context(tc.tile_pool(name="ktp", bufs=4))
        qtp = ctx.enter_context(tc.tile_pool(name="qtp", bufs=4))
        vp = ctx.enter_context(tc.tile_pool(name="vp", bufs=NI))
        KT = [ktp.tile([P, SEQ], FP, name=f"kt{m}", tag="kt") for m in range(4)]
        QT = [qtp.tile([P, SEQ], FP, name=f"qt{m}", tag="qt") for m in range(4)]
        V = [vp.tile([P, NH, DH + 1], FP, name=f"v{j}", tag="v") for j in range(NI)]

        # ---------------- projection phase ----------------
        with ExitStack() as pctx:
            actp = pctx.enter_context(tc.tile_pool(name="actp", bufs=NK))
            natp = pctx.enter_context(tc.tile_pool(name="natp", bufs=3))
            wp = pctx.enter_context(tc.tile_pool(name="wp", bufs=16))
            ppsum = pctx.enter_context(tc.tile_pool(name="ppsum", bufs=4, space="PSUM"))
            tpsum = pctx.enter_context(tc.tile_pool(name="tpsum", bufs=4, space="PSUM"))

            wk = [wp.tile([P, CC], FP, name=f"wk{k}", tag="w") for k in range(NK)]
            wv = [wp.tile([P, CC], FP, name=f"wv{k}", tag="w") for k in range(NK)]
            for k in range(NK):
                nc.sync.dma_start(out=wk[k], in_=wk_d[k * P:(k + 1) * P, :])
                nc.sync.dma_start(out=wv[k], in_=wv_d[k * P:(k + 1) * P, :])

            # context -> cT
            cT = [actp.tile([P, SEQ], FP, name=f"ct{k}", tag="act") for k in range(NK)]
            for i in range(NI):
                nat = natp.tile([P, DIM], FP, name="nat", tag="nat")
                nc.sync.dma_start(out=nat, in_=c_d[i * P:(i + 1) * P, :])
                for k in range(NK):
                    tp = tpsum.tile([P, P], FP, name="tp", tag="tp")
                    nc.tensor.transpose(tp, nat[:, k * P:(k + 1) * P], ident)
                    nc.vector.tensor_copy(cT[k][:, i * P:(i + 1) * P], tp)

            # KT[m][:, :] = sum_k wk[k][:, m*128:+128].T @ cT[k]
            for m in range(4):
                for i4 in range(4):
                    ps = ppsum.tile([P, 512], FP, name="ps", tag="ps")
                    for k in range(NK):
                        nc.tensor.matmul(
                            ps,
                            wk[k][:, m * P:(m + 1) * P],
                            cT[k][:, i4 * 512:(i4 + 1) * 512],
                            start=(k == 0),
                            stop=(k == NK - 1),
                        )
                    nc.vector.tensor_copy(KT[m][:, i4 * 512:(i4 + 1) * 512], ps)

            # V[j] = cT[:, j].T @ wv  (natural layout), plus ones column
            for j in range(NI):
                ps = ppsum.tile([P, 512], FP, name="psv", tag="ps")
                for k in range(NK):
                    nc.tensor.matmul(
                        ps,
                        cT[k][:, j * P:(j + 1) * P],
                        wv[k],
                        start=(k == 0),
                        stop=(k == NK - 1),
                    )
                nc.vector.tensor_copy(
                    V[j][:, :, 0:DH], ps.rearrange("p (h d) -> p h d", h=NH)
                )
                nc.vector.memset(V[j][:, :, DH:DH + 1], 1.0)

            # x -> xT (reuses cT slots)
            xT = [actp.tile([P, SEQ], FP, name=f"xt{k}", tag="act") for k in range(NK)]
            for i in range(NI):
                nat = natp.tile([P, DIM], FP, name="natx", tag="nat")
                nc.sync.dma_start(out=nat, in_=x_d[i * P:(i + 1) * P, :])
                for k in range(NK):
                    tp = tpsum.tile([P, P], FP, name="tpx", tag="tp")
                    nc.tensor.transpose(tp, nat[:, k * P:(k + 1) * P], ident)
                    nc.vector.tensor_copy(xT[k][:, i * P:(i + 1) * P], tp)

            wq = [wp.tile([P, CC], FP, name=f"wq{k}", tag="w") for k in range(NK)]
            for k in range(NK):
                nc.sync.dma_start(out=wq[k], in_=wq_d[k * P:(k + 1) * P, :])
            for m in range(4):
                for i4 in range(4):
                    ps = ppsum.tile([P, 512], FP, name="psq", tag="ps")
                    for k in range(NK):
                        nc.tensor.matmul(
                            ps,
                            wq[k][:, m * P:(m + 1) * P],
                            xT[k][:, i4 * 512:(i4 + 1) * 512],
                            start=(k == 0),
                            stop=(k == NK - 1),
                        )
                    nc.vector.tensor_copy(QT[m][:, i4 * 512:(i4 + 1) * 512], ps)

        # ---------------- attention phase ----------------
        with ExitStack() as actx:
            ptp = actx.enter_context(tc.tile_pool(name="ptp", bufs=3))
            outp = actx.enter_context(tc.tile_pool(name="outp", bufs=16))
            atsbp = actx.enter_context(tc.tile_pool(name="atsbp", bufs=2))
            recp = actx.enter_context(tc.tile_pool(name="recp", bufs=4))
            spsum = actx.enter_context(tc.tile_pool(name="spsum", bufs=2, space="PSUM"))
            apsum = actx.enter_context(tc.tile_pool(name="apsum", bufs=1, space="PSUM"))
            opsum = actx.enter_context(tc.tile_pool(name="opsum", bufs=2, space="PSUM"))

            for imac in range(NIM):
                outs = [
                    outp.tile([P, CC], FP, name=f"o{imac}_{b}", tag="o")
                    for b in range(IM // P)
                ]
                for h in range(NH):
                    kt = KT[h // 2]
                    qt = QT[h // 2]
                    po = (h % 2) * DH
                    at = apsum.tile([DH + 1, IM], FP, name="at", tag="at")
                    for j in range(NI):
                        sp = spsum.tile([P, IM], FP, name="sp", tag="sp")
                        for s in range(IM // 512):
                            nc.tensor.matmul(
                                sp[:, s * 512:(s + 1) * 512],
                                kt[po:po + DH, j * P:(j + 1) * P],
                                qt[po:po + DH,
                                   imac * IM + s * 512:imac * IM + (s + 1) * 512],
                                start=True,
                                stop=True,
                            )
                        pt = ptp.tile([P, IM], FP, name="pt", tag="pt")
                        nc.scalar.activation(pt, sp, EXP, scale=SCALE)
                        for s in range(IM // 512):
                            nc.tensor.matmul(
                                at[:, s * 512:(s + 1) * 512],
                                V[j][:, h, :],
                                pt[:, s * 512:(s + 1) * 512],
                                start=(j == 0),
                                stop=(j == NI - 1),
                            )
                    ats = atsbp.tile([DH + 1, IM], FP, name="ats", tag="ats")
                    nc.vector.tensor_copy(ats, at)
                    for blk in range(IM // P):
                        op = opsum.tile([P, DH + 1], FP, name="op", tag="op")
                        nc.tensor.transpose(
                            op, ats[:, blk * P:(blk + 1) * P],
                            ident[0:DH + 1, 0:DH + 1],
                        )
                        rec = recp.tile([P, 1], FP, name="rec", tag="rec")
                        nc.vector.reciprocal(rec, op[:, DH:DH + 1])
                        nc.vector.tensor_scalar_mul(
                            outs[blk][:, h * DH:(h + 1) * DH], op[:, 0:DH], rec
                        )
                for blk in range(IM // P):
                    i0 = imac * IM + blk * P
                    nc.sync.dma_start(out=out_d[i0:i0 + P, :], in_=outs[blk])


def _build():
    global _NC
    if _NC is not None:
        return _NC
    nc = bacc.Bacc(None, target_bir_lowering=False, debug=False)
    with TileContext(nc) as tc:
        with tc.tile_pool(name="dram", bufs=1, space="DRAM") as dram:
            x_d = dram.tile([SEQ, DIM], FP, kind="ExternalInput", name="x",
                            uniquify=False)
            c_d = dram.tile([SEQ, DIM], FP, kind="ExternalInput", name="ctx",
                            uniquify=False)
            wq_d = dram.tile([DIM, CC], FP, kind="ExternalInput", name="wq",
                             uniquify=False)
            wk_d = dram.tile([DIM, CC], FP, kind="ExternalInput", name="wk",
                             uniquify=False)
            wv_d = dram.tile([DIM, CC], FP, kind="ExternalInput", name="wv",
                             uniquify=False)
            out_d = dram.tile([SEQ, CC], FP, kind="ExternalOutput", name="out",
                              uniquify=False)
            _build_body(nc, tc, x_d, c_d, wq_d, wk_d, wv_d, out_d)
    nc.compile()
    _NC = nc
    return nc


def make_in_maps(x, context, Wq, Wkv):
    x = np.asarray(x, dtype=np.float32)
    context = np.asarray(context, dtype=np.float32)
    Wq = np.asarray(Wq, dtype=np.float32)
    Wkv = np.asarray(Wkv, dtype=np.float32)
    in_maps = []
    for core in range(8):
        b, hg = divmod(core, 2)
        c0 = hg * CC
        in_maps.append({
            "x": np.ascontiguousarray(x[b]),
            "ctx": np.ascontiguousarray(context[b]),
            "wq": np.ascontiguousarray(Wq[:, c0:c0 + CC]),
            "wk": np.ascontiguousarray(Wkv[:, c0:c0 + CC]),
            "wv": np.ascontiguousarray(Wkv[:, DIM + c0:DIM + c0 + CC]),
        })
    return in_maps


def run(x, context, Wq, Wkv, **run_kwargs):
    nc = _build()
    in_maps = make_in_maps(x, context, Wq, Wkv)
    res = run_bass_kernel_spmd(nc, in_maps, core_ids=list(range(8)), **run_kwargs)
    out = np.empty((4, SEQ, DIM), dtype=np.float32)
    for core in range(8):
        b, hg = divmod(core, 2)
        out[b, :, hg * CC:(hg + 1) * CC] = res.results[core]["out"]
    return out, res


def kernel(x, context, Wq, Wkv):
    out, _ = run(x, context, Wq, Wkv)
    return out


# revision 2
# speedup vs baseline: 1.9949x; 1.9949x over previous
"""Cross-attention kernel for 8 Trainium2 NeuronCores.

Contract: kernel(**inputs) takes FULL unsharded numpy inputs
(x [4,2048,1024], context [4,2048,1024], Wq [1024,1024], Wkv [1024,2048])
and returns the full output [4, 2048, 1024] (float32).

Sharding (hardcoded): core = b * 2 + hg handles batch b (0..3) and head
group hg (0..1) = heads hg*8 .. hg*8+7 (16 heads total, d=64). Data +
tensor parallel: no cross-core communication needed (softmax is per-row).

Matmuls run in bf16 (fp32 runs 2-pass LOW_HIGH on the PE = half
throughput); accumulation stays fp32 in PSUM. Inputs are cast to bf16 on
the host; softmax statistics and the output are fp32.

Per-core dataflow:
  cT = context[b].T              (PE transpose via identity, bf16)
  KT = Wk_slice.T @ cT           [512 c, 2048 j]  bf16
  V  = cT.T @ Wv_slice           [2048 j, 8 h, 65] bf16 (col 64 = 1.0)
  xT = x[b].T ; QT = Wq_slice.T @ xT   [512 c, 2048 i] bf16
  per (head h, i-macro of 1024):
    for j-chunk of 128:
      S^T  = K_h^T' Q_h^T        [128 j, 1024 i]  PSUM f32  (K=64 matmul)
      P^T  = exp(S^T / 8)        ACT, PSUM -> SBUF bf16 (no max-sub:
                                  scores ~ N(0,1), exp is range-safe)
      AT  += [V_h|1].T @ P^T     [65, 1024 i]  PSUM f32 accumulate
    AT -> SBUF, PE-transpose 128-col blocks -> [128 i, 65] PSUM
    out_sb[:, h*64:+64] = AT_t[:, :64] * recip(AT_t[:, 64])   (DVE)
  DMA out_sb -> out[2048, 512] f32 DRAM (host scatters into full output)
"""

import sys

if "/opt/trn_rl_repo" not in sys.path:
    sys.path.insert(0, "/opt/trn_rl_repo")

from contextlib import ExitStack

import ml_dtypes
import numpy as np

import concourse.bass as bass  # noqa: F401  (registers AP machinery)
import concourse.mybir as mybir
from concourse import bacc
from concourse.bass_utils import run_bass_kernel_spmd
from concourse.masks import make_identity
from concourse.tile import TileContext

FP = mybir.dt.float32
BF = mybir.dt.bfloat16
P = 128
SEQ = 2048
DIM = 1024
CC = 512  # per-core channel cols (8 heads x 64)
NH = 8  # heads per core
DH = 64  # head dim
NI = SEQ // P  # 16 seq chunks
NK = DIM // P  # 8 contraction chunks
IM = 1024  # i-macro width for attention
NIM = SEQ // IM  # 2
SCALE = DH ** -0.5

EXP = mybir.ActivationFunctionType.Exp

_NC = None


def _build_body(nc, tc, x_d, c_d, wq_d, wk_d, wv_d, out_d):
    with ExitStack() as ctx:
        const = ctx.enter_context(tc.tile_pool(name="const", bufs=1))
        ident = const.tile([P, P], BF, name="ident")
        make_identity(nc, ident)
        identf = const.tile([P, P], FP, name="identf")
        make_identity(nc, identf)

        # persistent across phases
        ktp = ctx.enter_context(tc.tile_pool(name="ktp", bufs=4))
        qtp = ctx.enter_context(tc.tile_pool(name="qtp", bufs=4))
        vp = ctx.enter_context(tc.tile_pool(name="vp", bufs=NI))
        KT = [ktp.tile([P, SEQ], BF, name=f"kt{m}", tag="kt") for m in range(4)]
        QT = [qtp.tile([P, SEQ], BF, name=f"qt{m}", tag="qt") for m in range(4)]
        V = [vp.tile([P, NH, DH + 1], BF, name=f"v{j}", tag="v") for j in range(NI)]

        # ---------------- projection phase ----------------
        with ExitStack() as pctx:
            actp = pctx.enter_context(tc.tile_pool(name="actp", bufs=NK))
            natp = pctx.enter_context(tc.tile_pool(name="natp", bufs=3))
            wp = pctx.enter_context(tc.tile_pool(name="wp", bufs=16))
            ppsum = pctx.enter_context(tc.tile_pool(name="ppsum", bufs=4, space="PSUM"))
            tpsum = pctx.enter_context(tc.tile_pool(name="tpsum", bufs=4, space="PSUM"))

            wk = [wp.tile([P, CC], BF, name=f"wk{k}", tag="w") for k in range(NK)]
            wv = [wp.tile([P, CC], BF, name=f"wv{k}", tag="w") for k in range(NK)]
            for k in range(NK):
                nc.sync.dma_start(out=wk[k], in_=wk_d[k * P:(k + 1) * P, :])
                nc.sync.dma_start(out=wv[k], in_=wv_d[k * P:(k + 1) * P, :])

            # context -> cT
            cT = [actp.tile([P, SEQ], BF, name=f"ct{k}", tag="act") for k in range(NK)]
            for i in range(NI):
                nat = natp.tile([P, DIM], BF, name="nat", tag="nat")
                nc.sync.dma_start(out=nat, in_=c_d[i * P:(i + 1) * P, :])
                for k in range(NK):
                    tp = tpsum.tile([P, P], BF, name="tp", tag="tp")
                    nc.tensor.transpose(tp, nat[:, k * P:(k + 1) * P], ident)
                    nc.vector.tensor_copy(cT[k][:, i * P:(i + 1) * P], tp)

            # KT[m][:, :] = sum_k wk[k][:, m*128:+128].T @ cT[k]
            for m in range(4):
                for i4 in range(4):
                    ps = ppsum.tile([P, 512], FP, name="ps", tag="ps")
                    for k in range(NK):
                        nc.tensor.matmul(
                            ps,
                            wk[k][:, m * P:(m + 1) * P],
                            cT[k][:, i4 * 512:(i4 + 1) * 512],
                            start=(k == 0),
                            stop=(k == NK - 1),
                        )
                    nc.vector.tensor_copy(KT[m][:, i4 * 512:(i4 + 1) * 512], ps)

            # V[j] = cT[:, j].T @ wv  (natural layout), plus ones column
            for j in range(NI):
                ps = ppsum.tile([P, 512], FP, name="psv", tag="ps")
                for k in range(NK):
                    nc.tensor.matmul(
                        ps,
                        cT[k][:, j * P:(j + 1) * P],
                        wv[k],
                        start=(k == 0),
                        stop=(k == NK - 1),
                    )
                nc.vector.tensor_copy(
                    V[j][:, :, 0:DH], ps.rearrange("p (h d) -> p h d", h=NH)
                )
                nc.vector.memset(V[j][:, :, DH:DH + 1], 1.0)

            # x -> xT (reuses cT slots)
            xT = [actp.tile([P, SEQ], BF, name=f"xt{k}", tag="act") for k in range(NK)]
            for i in range(NI):
                nat = natp.tile([P, DIM], BF, name="natx", tag="nat")
                nc.sync.dma_start(out=nat, in_=x_d[i * P:(i + 1) * P, :])
                for k in range(NK):
                    tp = tpsum.tile([P, P], BF, name="tpx", tag="tp")
                    nc.tensor.transpose(tp, nat[:, k * P:(k + 1) * P], ident)
                    nc.vector.tensor_copy(xT[k][:, i * P:(i + 1) * P], tp)

            wq = [wp.tile([P, CC], BF, name=f"wq{k}", tag="w") for k in range(NK)]
            for k in range(NK):
                nc.sync.dma_start(out=wq[k], in_=wq_d[k * P:(k + 1) * P, :])
            for m in range(4):
                for i4 in range(4):
                    ps = ppsum.tile([P, 512], FP, name="psq", tag="ps")
                    for k in range(NK):
                        nc.tensor.matmul(
                            ps,
                            wq[k][:, m * P:(m + 1) * P],
                            xT[k][:, i4 * 512:(i4 + 1) * 512],
                            start=(k == 0),
                            stop=(k == NK - 1),
                        )
                    nc.vector.tensor_copy(QT[m][:, i4 * 512:(i4 + 1) * 512], ps)

        # ---------------- attention phase ----------------
        with ExitStack() as actx:
            ptp = actx.enter_context(tc.tile_pool(name="ptp", bufs=3))
            outp = actx.enter_context(tc.tile_pool(name="outp", bufs=16))
            atsbp = actx.enter_context(tc.tile_pool(name="atsbp", bufs=2))
            recp = actx.enter_context(tc.tile_pool(name="recp", bufs=4))
            spsum = actx.enter_context(tc.tile_pool(name="spsum", bufs=2, space="PSUM"))
            apsum = actx.enter_context(tc.tile_pool(name="apsum", bufs=1, space="PSUM"))
            opsum = actx.enter_context(tc.tile_pool(name="opsum", bufs=2, space="PSUM"))

            for imac in range(NIM):
                outs = [
                    outp.tile([P, CC], FP, name=f"o{imac}_{b}", tag="o")
                    for b in range(IM // P)
                ]
                for h in range(NH):
                    kt = KT[h // 2]
                    qt = QT[h // 2]
                    po = (h % 2) * DH
                    at = apsum.tile([DH + 1, IM], FP, name="at", tag="at")
                    for j in range(NI):
                        sp = spsum.tile([P, IM], FP, name="sp", tag="sp")
                        for s in range(IM // 512):
                            nc.tensor.matmul(
                                sp[:, s * 512:(s + 1) * 512],
                                kt[po:po + DH, j * P:(j + 1) * P],
                                qt[po:po + DH,
                                   imac * IM + s * 512:imac * IM + (s + 1) * 512],
                                start=True,
                                stop=True,
                            )
                        pt = ptp.tile([P, IM], BF, name="pt", tag="pt")
                        nc.scalar.activation(pt, sp, EXP, scale=SCALE)
                        for s in range(IM // 512):
                            nc.tensor.matmul(
                                at[:, s * 512:(s + 1) * 512],
                                V[j][:, h, :],
                                pt[:, s * 512:(s + 1) * 512],
                                start=(j == 0),
                                stop=(j == NI - 1),
                            )
                    ats = atsbp.tile([DH + 1, IM], FP, name="ats", tag="ats")
                    nc.vector.tensor_copy(ats, at)
                    for blk in range(IM // P):
                        op = opsum.tile([P, DH + 1], FP, name="op", tag="op")
                        nc.tensor.transpose(
                            op, ats[:, blk * P:(blk + 1) * P],
                            identf[0:DH + 1, 0:DH + 1],
                        )
                        rec = recp.tile([P, 1], FP, name="rec", tag="rec")
                        nc.vector.reciprocal(rec, op[:, DH:DH + 1])
                        nc.vector.tensor_scalar_mul(
                            outs[blk][:, h * DH:(h + 1) * DH], op[:, 0:DH], rec
                        )
                for blk in range(IM // P):
                    i0 = imac * IM + blk * P
                    nc.sync.dma_start(out=out_d[i0:i0 + P, :], in_=outs[blk])


def _build():
    global _NC
    if _NC is not None:
        return _NC
    nc = bacc.Bacc(None, target_bir_lowering=False, debug=False)
    with TileContext(nc) as tc:
        with tc.tile_pool(name="dram", bufs=1, space="DRAM") as dram:
            x_d = dram.tile([SEQ, DIM], BF, kind="ExternalInput", name="x",
                            uniquify=False)
            c_d = dram.tile([SEQ, DIM], BF, kind="ExternalInput", name="ctx",
                            uniquify=False)
            wq_d = dram.tile([DIM, CC], BF, kind="ExternalInput", name="wq",
                             uniquify=False)
            wk_d = dram.tile([DIM, CC], BF, kind="ExternalInput", name="wk",
                             uniquify=False)
            wv_d = dram.tile([DIM, CC], BF, kind="ExternalInput", name="wv",
                             uniquify=False)
            out_d = dram.tile([SEQ, CC], FP, kind="ExternalOutput", name="out",
                              uniquify=False)
            _build_body(nc, tc, x_d, c_d, wq_d, wk_d, wv_d, out_d)
    nc.compile()
    _NC = nc
    return nc


def make_in_maps(x, context, Wq, Wkv):
    bf16 = ml_dtypes.bfloat16
    x = np.asarray(x, dtype=np.float32).astype(bf16)
    context = np.asarray(context, dtype=np.float32).astype(bf16)
    Wq = np.asarray(Wq, dtype=np.float32).astype(bf16)
    Wkv = np.asarray(Wkv, dtype=np.float32).astype(bf16)
    in_maps = []
    for core in range(8):
        b, hg = divmod(core, 2)
        c0 = hg * CC
        in_maps.append({
            "x": np.ascontiguousarray(x[b]),
            "ctx": np.ascontiguousarray(context[b]),
            "wq": np.ascontiguousarray(Wq[:, c0:c0 + CC]),
            "wk": np.ascontiguousarray(Wkv[:, c0:c0 + CC]),
            "wv": np.ascontiguousarray(Wkv[:, DIM + c0:DIM + c0 + CC]),
        })
    return in_maps


def run(x, context, Wq, Wkv, **run_kwargs):
    nc = _build()
    in_maps = make_in_maps(x, context, Wq, Wkv)
    res = run_bass_kernel_spmd(nc, in_maps, core_ids=list(range(8)), **run_kwargs)
    out = np.empty((4, SEQ, DIM), dtype=np.float32)
    for core in range(8):
        b, hg = divmod(core, 2)
        out[b, :, hg * CC:(hg + 1) * CC] = res.results[core]["out"]
    return out, res


def kernel(x, context, Wq, Wkv):
    out, _ = run(x, context, Wq, Wkv)
    return out


# revision 5
# speedup vs baseline: 2.7314x; 1.3692x over previous
"""Cross-attention kernel for 8 Trainium2 NeuronCores.

Contract: kernel(**inputs) takes FULL unsharded numpy inputs
(x [4,2048,1024], context [4,2048,1024], Wq [1024,1024], Wkv [1024,2048])
and returns the full output [4, 2048, 1024] (float32).

Sharding (hardcoded): core = b * 2 + hg handles batch b (0..3) and head
group hg (0..1) = heads hg*8 .. hg*8+7 (16 heads total, d=64). Data +
tensor parallel: no cross-core communication (softmax is per-row).

Matmuls run in bf16 (fp32 is 2-pass LOW_HIGH on the PE = half
throughput); accumulation is fp32 in PSUM. Inputs are cast to bf16 on
the host. Output is fp32.

Per-core dataflow:
  cT = context[b].T               (PE transpose, bf16)
  KT = Wk_slice.T @ cT            [512 c, 2048 j] bf16
  V  = cT.T @ Wv_slice            [2048 j, 8 h, 65] bf16 (col 64 = 1.0)
  xT = x[b].T ; QT = Wq_slice.T @ xT   [512 c, 2048 i] bf16
  per (head h, i-macro of 1024):
    for j-chunk of 128:
      S^T = K_h^T' Q_h^T          [128 j, 1024 i] PSUM f32 (K=64 matmul)
      P^T = exp(S^T / 8)          ACT, PSUM -> SBUF bf16 (no max-sub:
                                   scores ~ N(0,1), exp is range-safe)
      per i-chunk of 128 (8):     natural-form attention accumulate
        at[:, ic] += P^T[:, ic].T @ [V_h|1]    [128 i, 65] PSUM f32
                                   (8 accumulators packed into 2 banks)
    out_sb[:, h*64:+64] = at[..:64] * recip(at[.., 64])   (DVE, per ic)
  DMA out_sb -> out[2048, 512] f32 DRAM (host scatters into full out)

KT[1..3] and QT i-macro chunks are emitted just-in-time between heads so
the Tile scheduler has dependency-free PE work to fill the bubbles of
the ACT-gated attention loop (keeps the PE HAM warm at K=8).
"""

import sys

if "/opt/trn_rl_repo" not in sys.path:
    sys.path.insert(0, "/opt/trn_rl_repo")

from contextlib import ExitStack

import ml_dtypes
import numpy as np

import concourse.bass as bass  # noqa: F401  (registers AP machinery)
import concourse.mybir as mybir
from concourse import bacc
from concourse.bass_utils import run_bass_kernel_spmd
from concourse.masks import make_identity
from concourse.tile import TileContext

FP = mybir.dt.float32
BF = mybir.dt.bfloat16
P = 128
SEQ = 2048
DIM = 1024
CC = 512  # per-core channel cols (8 heads x 64)
NH = 8  # heads per core
DH = 64  # head dim
NI = SEQ // P  # 16 seq chunks
NK = DIM // P  # 8 contraction chunks
IM = 1024  # i-macro width for attention
NIM = SEQ // IM  # 2
NIC = IM // P  # 8 i-chunks per macro
SCALE = DH ** -0.5

EXP = mybir.ActivationFunctionType.Exp

_NC = None


def _build_body(nc, tc, x_d, c_d, wq_d, wk_d, wv_d, out_d):
    with ExitStack() as ctx:
        const = ctx.enter_context(tc.tile_pool(name="const", bufs=1))
        ident = const.tile([P, P], BF, name="ident")
        make_identity(nc, ident)

        ctp = ctx.enter_context(tc.tile_pool(name="ctp", bufs=NK))
        xtp = ctx.enter_context(tc.tile_pool(name="xtp", bufs=NK))
        ktp = ctx.enter_context(tc.tile_pool(name="ktp", bufs=4))
        qtp = ctx.enter_context(tc.tile_pool(name="qtp", bufs=4))
        vp = ctx.enter_context(tc.tile_pool(name="vp", bufs=NI))
        wp = ctx.enter_context(tc.tile_pool(name="wp", bufs=24))
        natp = ctx.enter_context(tc.tile_pool(name="natp", bufs=3))
        ptp = ctx.enter_context(tc.tile_pool(name="ptp", bufs=4))
        outp = ctx.enter_context(tc.tile_pool(name="outp", bufs=10))
        recp = ctx.enter_context(tc.tile_pool(name="recp", bufs=8))
        KT = [ktp.tile([P, SEQ], BF, name=f"kt{m}", tag="kt") for m in range(4)]
        QT = [qtp.tile([P, SEQ], BF, name=f"qt{m}", tag="qt") for m in range(4)]
        V = [vp.tile([P, NH, DH + 1], BF, name=f"v{j}", tag="v") for j in range(NI)]

        pools = {}

        def transpose_in(dst, src_d):
            # src_d [2048, 1024] DRAM -> dst: 8 SBUF tiles [128 k, 2048 i]
            for i in range(NI):
                nat = natp.tile([P, DIM], BF, name="nat", tag="nat")
                nc.sync.dma_start(out=nat, in_=src_d[i * P:(i + 1) * P, :])
                for k in range(NK):
                    tp = pools["tpsum"].tile([P, P], BF, name="tp", tag="tp")
                    nc.tensor.transpose(tp, nat[:, k * P:(k + 1) * P], ident)
                    nc.vector.tensor_copy(dst[k][:, i * P:(i + 1) * P], tp)

        def proj_chunk(dst, w, src, m, i4):
            # dst[m][:, i4*512:+512] = sum_k w[k][:, m-slice].T @ src[k][:, i4]
            ps = pools["ppsum"].tile([P, 512], FP, name="ps", tag="ps")
            for k in range(NK):
                nc.tensor.matmul(
                    ps,
                    w[k][:, m * P:(m + 1) * P],
                    src[k][:, i4 * 512:(i4 + 1) * 512],
                    start=(k == 0),
                    stop=(k == NK - 1),
                )
            nc.vector.tensor_copy(dst[m][:, i4 * 512:(i4 + 1) * 512], ps)

        # ---- weights ----
        wk = [wp.tile([P, CC], BF, name=f"wk{k}", tag="w") for k in range(NK)]
        wv = [wp.tile([P, CC], BF, name=f"wv{k}", tag="w") for k in range(NK)]
        wq = [wp.tile([P, CC], BF, name=f"wq{k}", tag="w") for k in range(NK)]
        for k in range(NK):
            nc.sync.dma_start(out=wk[k], in_=wk_d[k * P:(k + 1) * P, :])
            nc.sync.dma_start(out=wv[k], in_=wv_d[k * P:(k + 1) * P, :])
            nc.sync.dma_start(out=wq[k], in_=wq_d[k * P:(k + 1) * P, :])

        # ---- projections needed up-front ----
        cT = [ctp.tile([P, SEQ], BF, name=f"ct{k}", tag="act") for k in range(NK)]
        xT = [xtp.tile([P, SEQ], BF, name=f"xt{k}", tag="act2") for k in range(NK)]
        with ExitStack() as pctx:
            pools["ppsum"] = pctx.enter_context(
                tc.tile_pool(name="ppsumA", bufs=4, space="PSUM"))
            pools["tpsum"] = pctx.enter_context(
                tc.tile_pool(name="tpsum", bufs=4, space="PSUM"))

            transpose_in(cT, c_d)

            for i4 in range(4):
                proj_chunk(KT, wk, cT, 0, i4)

            # V[j] = cT[:, j].T @ wv  (natural layout), plus ones column
            for j in range(NI):
                ps = pools["ppsum"].tile([P, 512], FP, name="psv", tag="ps")
                for k in range(NK):
                    nc.tensor.matmul(
                        ps,
                        cT[k][:, j * P:(j + 1) * P],
                        wv[k],
                        start=(k == 0),
                        stop=(k == NK - 1),
                    )
                nc.vector.tensor_copy(
                    V[j][:, :, 0:DH], ps.rearrange("p (h d) -> p h d", h=NH)
                )
                nc.vector.memset(V[j][:, :, DH:DH + 1], 1.0)

            transpose_in(xT, x_d)

        # ---------------- attention (with just-in-time projections) -------
        pools["ppsum"] = ctx.enter_context(
            tc.tile_pool(name="ppsumB", bufs=2, space="PSUM"))
        spsum = ctx.enter_context(tc.tile_pool(name="spsum", bufs=2, space="PSUM"))
        apsum = ctx.enter_context(tc.tile_pool(name="apsum", bufs=1, space="PSUM"))

        qt_done = set()
        kt_done = {0}

        for imac in range(NIM):
            outs = [
                outp.tile([P, CC], FP, name=f"o{imac}_{b}", tag="o")
                for b in range(NIC)
            ]
            for h in range(NH):
                m = h // 2
                # just-in-time filler projections for upcoming heads
                if m + 1 < 4 and m + 1 not in kt_done:
                    kt_done.add(m + 1)
                    for i4 in range(4):
                        proj_chunk(KT, wk, cT, m + 1, i4)
                for i4 in (2 * imac, 2 * imac + 1):
                    if (m, i4) not in qt_done:
                        qt_done.add((m, i4))
                        proj_chunk(QT, wq, xT, m, i4)

                kt = KT[m]
                qt = QT[m]
                po = (h % 2) * DH
                at = apsum.tile([P, 2, 512], FP, name="at", tag="at")
                for j in range(NI):
                    sp = spsum.tile([P, IM], FP, name="sp", tag="sp")
                    for s in range(IM // 512):
                        nc.tensor.matmul(
                            sp[:, s * 512:(s + 1) * 512],
                            kt[po:po + DH, j * P:(j + 1) * P],
                            qt[po:po + DH,
                               imac * IM + s * 512:imac * IM + (s + 1) * 512],
                            start=True,
                            stop=True,
                        )
                    pt = ptp.tile([P, IM], BF, name="pt", tag="pt")
                    nc.scalar.activation(pt, sp, EXP, scale=SCALE)
                    for ic in range(NIC):
                        nc.tensor.matmul(
                            at[:, ic // 4, (ic % 4) * 65:(ic % 4) * 65 + 65],
                            pt[:, ic * P:(ic + 1) * P],
                            V[j][:, h, :],
                            start=(j == 0),
                            stop=(j == NI - 1),
                        )
                for ic in range(NIC):
                    blk = at[:, ic // 4, (ic % 4) * 65:(ic % 4) * 65 + 65]
                    rec = recp.tile([P, 1], FP, name="rec", tag="rec")
                    nc.vector.reciprocal(rec, blk[:, DH:DH + 1])
                    nc.vector.tensor_scalar_mul(
                        outs[ic][:, h * DH:(h + 1) * DH], blk[:, 0:DH], rec
                    )
            for blk in range(NIC):
                i0 = imac * IM + blk * P
                nc.sync.dma_start(out=out_d[i0:i0 + P, :], in_=outs[blk])


def _build():
    global _NC
    if _NC is not None:
        return _NC
    nc = bacc.Bacc(None, target_bir_lowering=False, debug=False)
    with TileContext(nc) as tc:
        with tc.tile_pool(name="dram", bufs=1, space="DRAM") as dram:
            x_d = dram.tile([SEQ, DIM], BF, kind="ExternalInput", name="x",
                            uniquify=False)
            c_d = dram.tile([SEQ, DIM], BF, kind="ExternalInput", name="ctx",
                            uniquify=False)
            wq_d = dram.tile([DIM, CC], BF, kind="ExternalInput", name="wq",
                             uniquify=False)
            wk_d = dram.tile([DIM, CC], BF, kind="ExternalInput", name="wk",
                             uniquify=False)
            wv_d = dram.tile([DIM, CC], BF, kind="ExternalInput", name="wv",
                             uniquify=False)
            out_d = dram.tile([SEQ, CC], FP, kind="ExternalOutput", name="out",
                              uniquify=False)
            _build_body(nc, tc, x_d, c_d, wq_d, wk_d, wv_d, out_d)
    nc.compile()
    _NC = nc
    return nc


def make_in_maps(x, context, Wq, Wkv):
    bf16 = ml_dtypes.bfloat16
    x = np.asarray(x, dtype=np.float32).astype(bf16)
    context = np.asarray(context, dtype=np.float32).astype(bf16)
    Wq = np.asarray(Wq, dtype=np.float32).astype(bf16)
    Wkv = np.asarray(Wkv, dtype=np.float32).astype(bf16)
    in_maps = []
    for core in range(8):
        b, hg = divmod(core, 2)
        c0 = hg * CC
        in_maps.append({
            "x": np.ascontiguousarray(x[b]),
            "ctx": np.ascontiguousarray(context[b]),
            "wq": np.ascontiguousarray(Wq[:, c0:c0 + CC]),
            "wk": np.ascontiguousarray(Wkv[:, c0:c0 + CC]),
            "wv": np.ascontiguousarray(Wkv[:, DIM + c0:DIM + c0 + CC]),
        })
    return in_maps


def run(x, context, Wq, Wkv, **run_kwargs):
    nc = _build()
    in_maps = make_in_maps(x, context, Wq, Wkv)
    res = run_bass_kernel_spmd(nc, in_maps, core_ids=list(range(8)), **run_kwargs)
    out = np.empty((4, SEQ, DIM), dtype=np.float32)
    for core in range(8):
        b, hg = divmod(core, 2)
        out[b, :, hg * CC:(hg + 1) * CC] = res.results[core]["out"]
    return out, res


def kernel(x, context, Wq, Wkv):
    out, _ = run(x, context, Wq, Wkv)
    return out


# revision 6
# speedup vs baseline: 2.7428x; 1.0042x over previous
"""Cross-attention kernel for 8 Trainium2 NeuronCores.

Contract: kernel(**inputs) takes FULL unsharded numpy inputs
(x [4,2048,1024], context [4,2048,1024], Wq [1024,1024], Wkv [1024,2048])
and returns the full output [4, 2048, 1024] (float32).

Sharding (hardcoded): core = b * 2 + hg handles batch b (0..3) and head
group hg (0..1) = heads hg*8 .. hg*8+7 (16 heads total, d=64). Data +
tensor parallel: no cross-core communication (softmax is per-row).

Matmuls run in bf16 (fp32 is 2-pass LOW_HIGH on the PE = half
throughput); accumulation is fp32 in PSUM. Inputs are cast to bf16 on
the host. Output is fp32.

Per-core dataflow:
  cT = context[b].T               (PE transpose, bf16)
  KT = Wk_slice.T @ cT            [512 c, 2048 j] bf16
  V  = cT.T @ Wv_slice            [2048 j, 8 h, 65] bf16 (col 64 = 1.0)
  xT = x[b].T ; QT = Wq_slice.T @ xT   [512 c, 2048 i] bf16
  per (head h, i-macro of 1024):
    for j-chunk of 128:
      S^T = K_h^T' Q_h^T          [128 j, 1024 i] PSUM f32 (K=64 matmul)
      P^T = exp(S^T / 8)          ACT, PSUM -> SBUF bf16 (no max-sub:
                                   scores ~ N(0,1), exp is range-safe)
      per i-chunk of 128 (8):     natural-form attention accumulate
        at[:, ic] += P^T[:, ic].T @ [V_h|1]    [128 i, 65] PSUM f32
                                   (8 accumulators packed into 2 banks)
    out_sb[:, h*64:+64] = at[..:64] * recip(at[.., 64])   (DVE, per ic)
  DMA out_sb -> out[2048, 512] f32 DRAM (host scatters into full out)

KT[1..3] and QT i-macro chunks are emitted just-in-time between heads so
the Tile scheduler has dependency-free PE work to fill the bubbles of
the ACT-gated attention loop (keeps the PE HAM warm at K=8).
"""

import sys

if "/opt/trn_rl_repo" not in sys.path:
    sys.path.insert(0, "/opt/trn_rl_repo")

from contextlib import ExitStack

import ml_dtypes
import numpy as np

import concourse.bass as bass  # noqa: F401  (registers AP machinery)
import concourse.mybir as mybir
from concourse import bacc
from concourse.bass_utils import run_bass_kernel_spmd
from concourse.masks import make_identity
from concourse.tile import TileContext

FP = mybir.dt.float32
BF = mybir.dt.bfloat16
P = 128
SEQ = 2048
DIM = 1024
CC = 512  # per-core channel cols (8 heads x 64)
NH = 8  # heads per core
DH = 64  # head dim
NI = SEQ // P  # 16 seq chunks
NK = DIM // P  # 8 contraction chunks
IM = 1024  # i-macro width for attention
NIM = SEQ // IM  # 2
NIC = IM // P  # 8 i-chunks per macro
SCALE = DH ** -0.5

EXP = mybir.ActivationFunctionType.Exp

_NC = None


def _build_body(nc, tc, x_d, c_d, wq_d, wk_d, wv_d, out_d):
    with ExitStack() as ctx:
        const = ctx.enter_context(tc.tile_pool(name="const", bufs=1))
        ident = const.tile([P, P], BF, name="ident")
        make_identity(nc, ident)

        ctp = ctx.enter_context(tc.tile_pool(name="ctp", bufs=NK))
        xtp = ctx.enter_context(tc.tile_pool(name="xtp", bufs=NK))
        ktp = ctx.enter_context(tc.tile_pool(name="ktp", bufs=4))
        qtp = ctx.enter_context(tc.tile_pool(name="qtp", bufs=4))
        vp = ctx.enter_context(tc.tile_pool(name="vp", bufs=NI))
        wp = ctx.enter_context(tc.tile_pool(name="wp", bufs=24))
        natp = ctx.enter_context(tc.tile_pool(name="natp", bufs=3))
        ptp = ctx.enter_context(tc.tile_pool(name="ptp", bufs=4))
        outp = ctx.enter_context(tc.tile_pool(name="outp", bufs=10))
        recp = ctx.enter_context(tc.tile_pool(name="recp", bufs=8))
        KT = [ktp.tile([P, SEQ], BF, name=f"kt{m}", tag="kt") for m in range(4)]
        QT = [qtp.tile([P, SEQ], BF, name=f"qt{m}", tag="qt") for m in range(4)]
        V = [vp.tile([P, NH, DH + 1], BF, name=f"v{j}", tag="v") for j in range(NI)]

        pools = {}

        def transpose_in(dst, src_d):
            # src_d [2048, 1024] DRAM -> dst: 8 SBUF tiles [128 k, 2048 i]
            for i in range(NI):
                nat = natp.tile([P, DIM], BF, name="nat", tag="nat")
                nc.sync.dma_start(out=nat, in_=src_d[i * P:(i + 1) * P, :])
                for k in range(NK):
                    tp = pools["tpsum"].tile([P, P], BF, name="tp", tag="tp")
                    nc.tensor.transpose(tp, nat[:, k * P:(k + 1) * P], ident)
                    nc.vector.tensor_copy(dst[k][:, i * P:(i + 1) * P], tp)

        def proj_chunk(dst, w, src, m, i4):
            # dst[m][:, i4*512:+512] = sum_k w[k][:, m-slice].T @ src[k][:, i4]
            ps = pools["ppsum"].tile([P, 512], FP, name="ps", tag="ps")
            for k in range(NK):
                nc.tensor.matmul(
                    ps,
                    w[k][:, m * P:(m + 1) * P],
                    src[k][:, i4 * 512:(i4 + 1) * 512],
                    start=(k == 0),
                    stop=(k == NK - 1),
                )
            nc.vector.tensor_copy(dst[m][:, i4 * 512:(i4 + 1) * 512], ps)

        # ---- weights ----
        wk = [wp.tile([P, CC], BF, name=f"wk{k}", tag="w") for k in range(NK)]
        wv = [wp.tile([P, CC], BF, name=f"wv{k}", tag="w") for k in range(NK)]
        wq = [wp.tile([P, CC], BF, name=f"wq{k}", tag="w") for k in range(NK)]
        for k in range(NK):
            nc.sync.dma_start(out=wk[k], in_=wk_d[k * P:(k + 1) * P, :])
            nc.sync.dma_start(out=wv[k], in_=wv_d[k * P:(k + 1) * P, :])
            nc.sync.dma_start(out=wq[k], in_=wq_d[k * P:(k + 1) * P, :])

        # ---- projections needed up-front ----
        cT = [ctp.tile([P, SEQ], BF, name=f"ct{k}", tag="act") for k in range(NK)]
        xT = [xtp.tile([P, SEQ], BF, name=f"xt{k}", tag="act2") for k in range(NK)]
        with ExitStack() as pctx:
            pools["ppsum"] = pctx.enter_context(
                tc.tile_pool(name="ppsumA", bufs=4, space="PSUM"))
            pools["tpsum"] = pctx.enter_context(
                tc.tile_pool(name="tpsum", bufs=4, space="PSUM"))

            transpose_in(cT, c_d)

            for i4 in range(4):
                proj_chunk(KT, wk, cT, 0, i4)

            # V[j] = cT[:, j].T @ wv  (natural layout), plus ones column
            for j in range(NI):
                ps = pools["ppsum"].tile([P, 512], FP, name="psv", tag="ps")
                for k in range(NK):
                    nc.tensor.matmul(
                        ps,
                        cT[k][:, j * P:(j + 1) * P],
                        wv[k],
                        start=(k == 0),
                        stop=(k == NK - 1),
                    )
                nc.vector.tensor_copy(
                    V[j][:, :, 0:DH], ps.rearrange("p (h d) -> p h d", h=NH)
                )
                nc.vector.memset(V[j][:, :, DH:DH + 1], 1.0)

            transpose_in(xT, x_d)

        # ---------------- attention (with just-in-time projections) -------
        pools["ppsum"] = ctx.enter_context(
            tc.tile_pool(name="ppsumB", bufs=2, space="PSUM"))
        spsum = ctx.enter_context(tc.tile_pool(name="spsum", bufs=2, space="PSUM"))
        apsum = ctx.enter_context(tc.tile_pool(name="apsum", bufs=1, space="PSUM"))

        qt_done = set()
        kt_done = {0}

        for imac in range(NIM):
            outs = [
                outp.tile([P, CC], FP, name=f"o{imac}_{b}", tag="o")
                for b in range(NIC)
            ]
            for h in range(NH):
                m = h // 2
                # just-in-time filler projections for upcoming heads
                if m + 1 < 4 and m + 1 not in kt_done:
                    kt_done.add(m + 1)
                    for i4 in range(4):
                        proj_chunk(KT, wk, cT, m + 1, i4)
                for i4 in (2 * imac, 2 * imac + 1):
                    if (m, i4) not in qt_done:
                        qt_done.add((m, i4))
                        proj_chunk(QT, wq, xT, m, i4)

                kt = KT[m]
                qt = QT[m]
                po = (h % 2) * DH
                at = apsum.tile([P, 2, 512], FP, name="at", tag="at")
                for j in range(NI):
                    sp = spsum.tile([P, IM], FP, name="sp", tag="sp")
                    for s in range(IM // 512):
                        nc.tensor.matmul(
                            sp[:, s * 512:(s + 1) * 512],
                            kt[po:po + DH, j * P:(j + 1) * P],
                            qt[po:po + DH,
                               imac * IM + s * 512:imac * IM + (s + 1) * 512],
                            start=True,
                            stop=True,
                        )
                    pt = ptp.tile([P, IM], BF, name="pt", tag="pt")
                    nc.scalar.activation(pt, sp, EXP, scale=SCALE)
                    for ic in range(NIC):
                        # start=True clears the whole PSUM bank, so only
                        # the first matmul touching each bank may set it;
                        # later groups overwrite via cleared has_written.
                        nc.tensor.matmul(
                            at[:, ic // 4, (ic % 4) * 65:(ic % 4) * 65 + 65],
                            pt[:, ic * P:(ic + 1) * P],
                            V[j][:, h, :],
                            start=(j == 0 and ic % 4 == 0),
                            stop=(j == NI - 1 and ic % 4 == 3),
                            skip_group_check=True,
                        )
                for ic in range(NIC):
                    blk = at[:, ic // 4, (ic % 4) * 65:(ic % 4) * 65 + 65]
                    rec = recp.tile([P, 1], FP, name="rec", tag="rec")
                    nc.vector.reciprocal(rec, blk[:, DH:DH + 1])
                    nc.vector.tensor_scalar_mul(
                        outs[ic][:, h * DH:(h + 1) * DH], blk[:, 0:DH], rec
                    )
            for blk in range(NIC):
                i0 = imac * IM + blk * P
                nc.sync.dma_start(out=out_d[i0:i0 + P, :], in_=outs[blk])


def _build():
    global _NC
    if _NC is not None:
        return _NC
    nc = bacc.Bacc(None, target_bir_lowering=False, debug=False)
    with TileContext(nc) as tc:
        with tc.tile_pool(name="dram", bufs=1, space="DRAM") as dram:
            x_d = dram.tile([SEQ, DIM], BF, kind="ExternalInput", name="x",
                            uniquify=False)
            c_d = dram.tile([SEQ, DIM], BF, kind="ExternalInput", name="ctx",
                            uniquify=False)
            wq_d = dram.tile([DIM, CC], BF, kind="ExternalInput", name="wq",
                             uniquify=False)
            wk_d = dram.tile([DIM, CC], BF, kind="ExternalInput", name="wk",
                             uniquify=False)
            wv_d = dram.tile([DIM, CC], BF, kind="ExternalInput", name="wv",
                             uniquify=False)
            out_d = dram.tile([SEQ, CC], FP, kind="ExternalOutput", name="out",
                              uniquify=False)
            _build_body(nc, tc, x_d, c_d, wq_d, wk_d, wv_d, out_d)
    nc.compile()
    _NC = nc
    return nc


def make_in_maps(x, context, Wq, Wkv):
    bf16 = ml_dtypes.bfloat16
    x = np.asarray(x, dtype=np.float32).astype(bf16)
    context = np.asarray(context, dtype=np.float32).astype(bf16)
    Wq = np.asarray(Wq, dtype=np.float32).astype(bf16)
    Wkv = np.asarray(Wkv, dtype=np.float32).astype(bf16)
    in_maps = []
    for core in range(8):
        b, hg = divmod(core, 2)
        c0 = hg * CC
        in_maps.append({
            "x": np.ascontiguousarray(x[b]),
            "ctx": np.ascontiguousarray(context[b]),
            "wq": np.ascontiguousarray(Wq[:, c0:c0 + CC]),
            "wk": np.ascontiguousarray(Wkv[:, c0:c0 + CC]),
            "wv": np.ascontiguousarray(Wkv[:, DIM + c0:DIM + c0 + CC]),
        })
    return in_maps


def run(x, context, Wq, Wkv, **run_kwargs):
    nc = _build()
    in_maps = make_in_maps(x, context, Wq, Wkv)
    res = run_bass_kernel_spmd(nc, in_maps, core_ids=list(range(8)), **run_kwargs)
    out = np.empty((4, SEQ, DIM), dtype=np.float32)
    for core in range(8):
        b, hg = divmod(core, 2)
        out[b, :, hg * CC:(hg + 1) * CC] = res.results[core]["out"]
    return out, res


def kernel(x, context, Wq, Wkv):
    out, _ = run(x, context, Wq, Wkv)
    return out


# revision 9
# speedup vs baseline: 2.8175x; 1.0272x over previous
"""Cross-attention kernel for 8 Trainium2 NeuronCores.

Contract: kernel(**inputs) takes FULL unsharded numpy inputs
(x [4,2048,1024], context [4,2048,1024], Wq [1024,1024], Wkv [1024,2048])
and returns the full output [4, 2048, 1024] (float32).

Sharding (hardcoded): core = b * 2 + hg handles batch b (0..3) and head
group hg (0..1) = heads hg*8 .. hg*8+7 (16 heads total, d=64). Data +
tensor parallel: no cross-core communication (softmax is per-row).

Matmuls run in bf16 (fp32 is 2-pass LOW_HIGH on the PE = half
throughput); accumulation is fp32 in PSUM. Inputs are cast to bf16 on
the host. Output is fp32.

Per-core dataflow:
  cT = context[b].T               (PE transpose, bf16)
  KT = Wk_slice.T @ cT            [512 c, 2048 j] bf16
  V  = cT.T @ Wv_slice            [2048 j, 8 h, 65] bf16 (col 64 = 1.0)
  xT = x[b].T ; QT = Wq_slice.T @ xT   [512 c, 2048 i] bf16
  per (head h, i-macro of 1024):
    for j-chunk of 128:
      S^T = K_h^T' Q_h^T          [128 j, 1024 i] PSUM f32 (K=64 matmul)
      P^T = exp(S^T / 8)          ACT, PSUM -> SBUF bf16 (no max-sub:
                                   scores ~ N(0,1), exp is range-safe)
      per i-chunk of 128 (8):     natural-form attention accumulate
        at[:, ic] += P^T[:, ic].T @ [V_h|1]    [128 i, 65] PSUM f32
                                   (8 accumulators packed into 2 banks;
                                    start=True clears a whole bank, so
                                    only the first group per bank sets it)
    out_sb[:, h*64:+64] = at[..:64] * recip(at[.., 64])   (DVE, per ic)
  DMA out_sb -> out[2048, 512] f32 DRAM (host scatters into full out)

The attention inner loop is gated by ScalarE (exp); to keep the PE's
HAM governor warm (K=8), half the xT transposes, KT[1..3], and all QT
projection chunks are emitted as a metered filler queue between heads,
giving the scheduler dependency-free PE work for every bubble.
"""

import sys

if "/opt/trn_rl_repo" not in sys.path:
    sys.path.insert(0, "/opt/trn_rl_repo")

from contextlib import ExitStack

import ml_dtypes
import numpy as np

import concourse.bass as bass  # noqa: F401  (registers AP machinery)
import concourse.mybir as mybir
from concourse import bacc
from concourse.bass_utils import run_bass_kernel_spmd
from concourse.masks import make_identity
from concourse.tile import TileContext

FP = mybir.dt.float32
BF = mybir.dt.bfloat16
P = 128
SEQ = 2048
DIM = 1024
CC = 512  # per-core channel cols (8 heads x 64)
NH = 8  # heads per core
DH = 64  # head dim
NI = SEQ // P  # 16 seq chunks
NK = DIM // P  # 8 contraction chunks
IM = 1024  # i-macro width for attention
NIM = SEQ // IM  # 2
NIC = IM // P  # 8 i-chunks per macro
SCALE = DH ** -0.5

EXP = mybir.ActivationFunctionType.Exp

_NC = None


def _build_body(nc, tc, x_d, c_d, wq_d, wk_d, wv_d, out_d):
    with ExitStack() as ctx:
        const = ctx.enter_context(tc.tile_pool(name="const", bufs=1))
        ident = const.tile([P, P], BF, name="ident")
        make_identity(nc, ident)

        ctp = ctx.enter_context(tc.tile_pool(name="ctp", bufs=NK))
        xtp = ctx.enter_context(tc.tile_pool(name="xtp", bufs=NK))
        ktp = ctx.enter_context(tc.tile_pool(name="ktp", bufs=4))
        qtp = ctx.enter_context(tc.tile_pool(name="qtp", bufs=4))
        vp = ctx.enter_context(tc.tile_pool(name="vp", bufs=NI))
        wp = ctx.enter_context(tc.tile_pool(name="wp", bufs=24))
        natp = ctx.enter_context(tc.tile_pool(name="natp", bufs=4))
        ptp = ctx.enter_context(tc.tile_pool(name="ptp", bufs=4))
        outp = ctx.enter_context(tc.tile_pool(name="outp", bufs=10))
        recp = ctx.enter_context(tc.tile_pool(name="recp", bufs=8))
        # PSUM budget (8 banks): sp 2x2 + at 1x2 + fill 2x1 = 8
        fillp = ctx.enter_context(tc.tile_pool(name="fillp", bufs=2, space="PSUM"))
        spsum = ctx.enter_context(tc.tile_pool(name="spsum", bufs=2, space="PSUM"))
        apsum = ctx.enter_context(tc.tile_pool(name="apsum", bufs=1, space="PSUM"))

        KT = [ktp.tile([P, SEQ], BF, name=f"kt{m}", tag="kt") for m in range(4)]
        QT = [qtp.tile([P, SEQ], BF, name=f"qt{m}", tag="qt") for m in range(4)]
        V = [vp.tile([P, NH, DH + 1], BF, name=f"v{j}", tag="v") for j in range(NI)]
        cT = [ctp.tile([P, SEQ], BF, name=f"ct{k}", tag="act") for k in range(NK)]
        xT = [xtp.tile([P, SEQ], BF, name=f"xt{k}", tag="act2") for k in range(NK)]

        def evict(dst, src):
            nc.vector.tensor_copy(dst, src)

        def transpose_chunk(dst, src_d, i):
            # one [128, 1024] row block of src -> 8 [128,128] blocks of dst
            nat = natp.tile([P, DIM], BF, name="nat", tag="nat")
            nc.sync.dma_start(out=nat, in_=src_d[i * P:(i + 1) * P, :])
            for k in range(NK):
                tp = fillp.tile([P, P], BF, name="tp", tag="fp")
                nc.tensor.transpose(tp, nat[:, k * P:(k + 1) * P], ident)
                evict(dst[k][:, i * P:(i + 1) * P], tp)

        def proj_chunk(dst, w, src, m, i4):
            # dst[m][:, i4*512:+512] = sum_k w[k][:, m-slice].T @ src[k][:, i4]
            ps = fillp.tile([P, 512], FP, name="ps", tag="fp")
            for k in range(NK):
                nc.tensor.matmul(
                    ps,
                    w[k][:, m * P:(m + 1) * P],
                    src[k][:, i4 * 512:(i4 + 1) * 512],
                    start=(k == 0),
                    stop=(k == NK - 1),
                )
            nc.vector.tensor_copy(dst[m][:, i4 * 512:(i4 + 1) * 512], ps)

        # ---- up-front projections ----
        for i in range(NI):
            transpose_chunk(cT, c_d, i)

        wk = [wp.tile([P, CC], BF, name=f"wk{k}", tag="w") for k in range(NK)]
        wv = [wp.tile([P, CC], BF, name=f"wv{k}", tag="w") for k in range(NK)]
        wq = [wp.tile([P, CC], BF, name=f"wq{k}", tag="w") for k in range(NK)]
        for k in range(NK):
            nc.sync.dma_start(out=wk[k], in_=wk_d[k * P:(k + 1) * P, :])
            nc.sync.dma_start(out=wv[k], in_=wv_d[k * P:(k + 1) * P, :])
            nc.sync.dma_start(out=wq[k], in_=wq_d[k * P:(k + 1) * P, :])

        for i4 in range(4):
            proj_chunk(KT, wk, cT, 0, i4)

        # V[j] = cT[:, j].T @ wv  (natural layout), plus ones column
        for j in range(NI):
            ps = fillp.tile([P, 512], FP, name="psv", tag="fp")
            for k in range(NK):
                nc.tensor.matmul(
                    ps,
                    cT[k][:, j * P:(j + 1) * P],
                    wv[k],
                    start=(k == 0),
                    stop=(k == NK - 1),
                )
            nc.vector.tensor_copy(
                V[j][:, :, 0:DH], ps.rearrange("p (h d) -> p h d", h=NH)
            )
            nc.vector.memset(V[j][:, :, DH:DH + 1], 1.0)

        # first half of xT (feeds QT i4=0,1 used by i-macro 0)
        for i in range(NIC):
            transpose_chunk(xT, x_d, i)
        proj_chunk(QT, wq, xT, 0, 0)
        proj_chunk(QT, wq, xT, 0, 1)

        # ---- filler queue: emitted between heads to keep the PE dense ----
        # (im, h) -> list of thunks; deadlines: KT[m] before head 2m,
        # xT i-chunks 8-15 + QT[m][:, im1] before (im=1, h=2m).
        filler = {
            (0, 1): [lambda m=1, i4=i4: proj_chunk(KT, wk, cT, m, i4)
                     for i4 in range(4)]
                    + [lambda i4=i4: proj_chunk(QT, wq, xT, 1, i4)
                       for i4 in range(2)],
            (0, 3): [lambda m=2, i4=i4: proj_chunk(KT, wk, cT, m, i4)
                     for i4 in range(4)]
                    + [lambda i4=i4: proj_chunk(QT, wq, xT, 2, i4)
                       for i4 in range(2)],
            (0, 5): [lambda m=3, i4=i4: proj_chunk(KT, wk, cT, m, i4)
                     for i4 in range(4)]
                    + [lambda i4=i4: proj_chunk(QT, wq, xT, 3, i4)
                       for i4 in range(2)]
                    + [lambda i=8: transpose_chunk(xT, x_d, i),
                       lambda i=9: transpose_chunk(xT, x_d, i)],
            (0, 6): [lambda i=i: transpose_chunk(xT, x_d, i)
                     for i in range(10, 13)],
            (0, 7): [lambda i=i: transpose_chunk(xT, x_d, i)
                     for i in range(13, 16)]
                    + [lambda: proj_chunk(QT, wq, xT, 0, 2),
                       lambda: proj_chunk(QT, wq, xT, 0, 3)],
            (1, 0): [lambda: proj_chunk(QT, wq, xT, 1, 2)],
            (1, 1): [lambda: proj_chunk(QT, wq, xT, 1, 3)],
            (1, 2): [lambda: proj_chunk(QT, wq, xT, 2, 2)],
            (1, 3): [lambda: proj_chunk(QT, wq, xT, 2, 3)],
            (1, 4): [lambda: proj_chunk(QT, wq, xT, 3, 2)],
            (1, 5): [lambda: proj_chunk(QT, wq, xT, 3, 3)],
        }

        # ---------------- attention ----------------
        for imac in range(NIM):
            outs = [
                outp.tile([P, CC], FP, name=f"o{imac}_{b}", tag="o")
                for b in range(NIC)
            ]
            for h in range(NH):
                for thunk in filler.get((imac, h), ()):
                    thunk()
                m = h // 2
                kt = KT[m]
                qt = QT[m]
                po = (h % 2) * DH
                at = apsum.tile([P, 2, 512], FP, name="at", tag="at")
                for j in range(NI):
                    sp = spsum.tile([P, IM], FP, name="sp", tag="sp")
                    for s in range(IM // 512):
                        nc.tensor.matmul(
                            sp[:, s * 512:(s + 1) * 512],
                            kt[po:po + DH, j * P:(j + 1) * P],
                            qt[po:po + DH,
                               imac * IM + s * 512:imac * IM + (s + 1) * 512],
                            start=True,
                            stop=True,
                        )
                    pt = ptp.tile([P, IM], BF, name="pt", tag="pt")
                    nc.scalar.activation(pt, sp, EXP, scale=SCALE)
                    for ic in range(NIC):
                        nc.tensor.matmul(
                            at[:, ic // 4, (ic % 4) * 65:(ic % 4) * 65 + 65],
                            pt[:, ic * P:(ic + 1) * P],
                            V[j][:, h, :],
                            start=(j == 0 and ic % 4 == 0),
                            stop=(j == NI - 1 and ic % 4 == 3),
                            skip_group_check=True,
                        )
                for ic in range(NIC):
                    blk = at[:, ic // 4, (ic % 4) * 65:(ic % 4) * 65 + 65]
                    rec = recp.tile([P, 1], FP, name="rec", tag="rec")
                    nc.vector.reciprocal(rec, blk[:, DH:DH + 1])
                    nc.vector.tensor_scalar_mul(
                        outs[ic][:, h * DH:(h + 1) * DH], blk[:, 0:DH], rec
                    )
            for blk in range(NIC):
                i0 = imac * IM + blk * P
                nc.sync.dma_start(out=out_d[i0:i0 + P, :], in_=outs[blk])


def _build():
    global _NC
    if _NC is not None:
        return _NC
    nc = bacc.Bacc(None, target_bir_lowering=False, debug=False)
    with TileContext(nc) as tc:
        with tc.tile_pool(name="dram", bufs=1, space="DRAM") as dram:
            x_d = dram.tile([SEQ, DIM], BF, kind="ExternalInput", name="x",
                            uniquify=False)
            c_d = dram.tile([SEQ, DIM], BF, kind="ExternalInput", name="ctx",
                            uniquify=False)
            wq_d = dram.tile([DIM, CC], BF, kind="ExternalInput", name="wq",
                             uniquify=False)
            wk_d = dram.tile([DIM, CC], BF, kind="ExternalInput", name="wk",
                             uniquify=False)
            wv_d = dram.tile([DIM, CC], BF, kind="ExternalInput", name="wv",
                             uniquify=False)
            out_d = dram.tile([SEQ, CC], FP, kind="ExternalOutput", name="out",
                              uniquify=False)
            _build_body(nc, tc, x_d, c_d, wq_d, wk_d, wv_d, out_d)
    nc.compile()
    _NC = nc
    return nc


def make_in_maps(x, context, Wq, Wkv):
    bf16 = ml_dtypes.bfloat16
    x = np.asarray(x, dtype=np.float32).astype(bf16)
    context = np.asarray(context, dtype=np.float32).astype(bf16)
    Wq = np.asarray(Wq, dtype=np.float32).astype(bf16)
    Wkv = np.asarray(Wkv, dtype=np.float32).astype(bf16)
    in_maps = []
    for core in range(8):
        b, hg = divmod(core, 2)
        c0 = hg * CC
        in_maps.append({
            "x": np.ascontiguousarray(x[b]),
            "ctx": np.ascontiguousarray(context[b]),
            "wq": np.ascontiguousarray(Wq[:, c0:c0 + CC]),
            "wk": np.ascontiguousarray(Wkv[:, c0:c0 + CC]),
            "wv": np.ascontiguousarray(Wkv[:, DIM + c0:DIM + c0 + CC]),
        })
    return in_maps


def run(x, context, Wq, Wkv, **run_kwargs):
    nc = _build()
    in_maps = make_in_maps(x, context, Wq, Wkv)
    res = run_bass_kernel_spmd(nc, in_maps, core_ids=list(range(8)), **run_kwargs)
    out = np.empty((4, SEQ, DIM), dtype=np.float32)
    for core in range(8):
        b, hg = divmod(core, 2)
        out[b, :, hg * CC:(hg + 1) * CC] = res.results[core]["out"]
    return out, res


def kernel(x, context, Wq, Wkv):
    out, _ = run(x, context, Wq, Wkv)
    return out


# revision 12
# speedup vs baseline: 3.0258x; 1.0739x over previous
"""Cross-attention kernel for 8 Trainium2 NeuronCores.

Contract: kernel(**inputs) takes FULL unsharded numpy inputs
(x [4,2048,1024], context [4,2048,1024], Wq [1024,1024], Wkv [1024,2048])
and returns the full output [4, 2048, 1024] (float32).

Sharding (hardcoded): core = b * 2 + hg handles batch b (0..3) and head
group hg (0..1) = heads hg*8 .. hg*8+7 (16 heads total, d=64). Data +
tensor parallel: no cross-core communication (softmax is per-row).

Matmuls run in bf16 (fp32 is 2-pass LOW_HIGH on the PE = half
throughput); accumulation is fp32 in PSUM. Inputs are cast to bf16 on
the host. Output is fp32.

Per-core dataflow:
  cT = context[b].T               (PE transpose, bf16)
  KT = Wk_slice.T @ cT            [512 c, 2048 j] bf16
  V  = cT.T @ Wv_slice            [2048 j, 8 h, 65] bf16 (col 64 = 1.0)
  xT = x[b].T ; QT = Wq_slice.T @ xT   [512 c, 2048 i] bf16
  per (head h, i-macro of 1024):
    for j-chunk of 128:
      S^T = K_h^T' Q_h^T          [128 j, 1024 i] PSUM f32 (K=64 matmul)
      P^T = exp(S^T / 8)          ACT, PSUM -> SBUF bf16 (no max-sub:
                                   scores ~ N(0,1), exp is range-safe)
      per i-chunk of 128 (8):     natural-form attention accumulate
        at[:, ic] += P^T[:, ic].T @ [V_h|1]    [128 i, 65] PSUM f32
                                   (8 accumulators packed into 2 banks;
                                    start=True clears a whole bank, so
                                    only the first group per bank sets it)
    out_sb[:, h*64:+64] = at[..:64] * recip(at[.., 64])   (DVE, per ic)
  DMA out_sb -> out[2048, 512] f32 DRAM (host scatters into full out)

The attention inner loop is gated by ScalarE (exp); to keep the PE's
HAM governor warm (K=8), half the xT transposes, KT[1..3], and all QT
projection chunks are emitted as a metered filler queue between heads,
giving the scheduler dependency-free PE work for every bubble.
"""

import sys

if "/opt/trn_rl_repo" not in sys.path:
    sys.path.insert(0, "/opt/trn_rl_repo")

from contextlib import ExitStack

import ml_dtypes
import numpy as np

import concourse.bass as bass  # noqa: F401  (registers AP machinery)
import concourse.mybir as mybir
from concourse import bacc
from concourse.bass_utils import run_bass_kernel_spmd
from concourse.masks import make_identity
from concourse.tile import TileContext

FP = mybir.dt.float32
BF = mybir.dt.bfloat16
P = 128
SEQ = 2048
DIM = 1024
CC = 512  # per-core channel cols (8 heads x 64)
NH = 8  # heads per core
DH = 64  # head dim
NI = SEQ // P  # 16 seq chunks
NK = DIM // P  # 8 contraction chunks
IM = 1024  # i-macro width for attention
NIM = SEQ // IM  # 2
NIC = IM // P  # 8 i-chunks per macro
SCALE = DH ** -0.5

EXP = mybir.ActivationFunctionType.Exp

_NC = None


def _build_body(nc, tc, x_d, c_d, wq_d, wk_d, wv_d, out_d):
    with ExitStack() as ctx:
        const = ctx.enter_context(tc.tile_pool(name="const", bufs=1))
        ident = const.tile([P, P], BF, name="ident")
        make_identity(nc, ident)

        ctp = ctx.enter_context(tc.tile_pool(name="ctp", bufs=1))
        xtp = ctx.enter_context(tc.tile_pool(name="xtp", bufs=1))
        ktp = ctx.enter_context(tc.tile_pool(name="ktp", bufs=4))
        qtp = ctx.enter_context(tc.tile_pool(name="qtp", bufs=4))
        vp = ctx.enter_context(tc.tile_pool(name="vp", bufs=NI))
        wp = ctx.enter_context(tc.tile_pool(name="wp", bufs=24))
        natp = ctx.enter_context(tc.tile_pool(name="natp", bufs=4))
        ptp = ctx.enter_context(tc.tile_pool(name="ptp", bufs=4))
        outp = ctx.enter_context(tc.tile_pool(name="outp", bufs=10))
        recp = ctx.enter_context(tc.tile_pool(name="recp", bufs=8))
        # PSUM budget (8 banks): sp 2x2 + at 1x2 + fill 2x1 = 8
        fillp = ctx.enter_context(tc.tile_pool(name="fillp", bufs=2, space="PSUM"))
        spsum = ctx.enter_context(tc.tile_pool(name="spsum", bufs=2, space="PSUM"))
        apsum = ctx.enter_context(tc.tile_pool(name="apsum", bufs=1, space="PSUM"))

        KT = [ktp.tile([P, SEQ], BF, name=f"kt{m}", tag="kt") for m in range(4)]
        QT = [qtp.tile([P, SEQ], BF, name=f"qt{m}", tag="qt") for m in range(4)]
        V = [vp.tile([P, NH, DH + 1], BF, name=f"v{j}", tag="v") for j in range(NI)]
        # consolidated transposed activations: [:, k, :] is the k-th
        # 128-row contraction slice (lets 4 transposes share one eviction)
        cT = ctp.tile([P, NK, SEQ], BF, name="ct", tag="act")
        xT = xtp.tile([P, NK, SEQ], BF, name="xt", tag="act2")

        def transpose_chunk(dst, src_d, i):
            # one [128, 1024] row block of src -> dst[:, :, i*128:+128];
            # 4 transposes share a PSUM bank (only the first may set
            # start: start=True clears the whole bank) and one eviction.
            nat = natp.tile([P, DIM], BF, name="nat", tag="nat")
            nc.sync.dma_start(out=nat, in_=src_d[i * P:(i + 1) * P, :])
            for half in range(2):
                tp = fillp.tile([P, 512], BF, name="tp", tag="fp")
                for q in range(4):
                    k = half * 4 + q
                    nc.tensor.matmul(
                        tp[:, q * P:(q + 1) * P],
                        nat[:, k * P:(k + 1) * P],
                        ident,
                        is_transpose=True,
                        start=(q == 0),
                        stop=(q == 3),
                        skip_group_check=True,
                    )
                nc.vector.tensor_copy(
                    dst[:, half * 4:half * 4 + 4, i * P:(i + 1) * P],
                    tp.rearrange("p (k c) -> p k c", k=4),
                )

        def proj_chunk(dst, w, src, m, i4):
            # dst[m][:, i4*512:+512] = sum_k w[k][:, m-slice].T @ src[:, k, i4]
            ps = fillp.tile([P, 512], FP, name="ps", tag="fp")
            for k in range(NK):
                nc.tensor.matmul(
                    ps,
                    w[k][:, m * P:(m + 1) * P],
                    src[:, k, i4 * 512:(i4 + 1) * 512],
                    start=(k == 0),
                    stop=(k == NK - 1),
                )
            nc.vector.tensor_copy(dst[m][:, i4 * 512:(i4 + 1) * 512], ps)

        def v_chunk(j):
            ps = fillp.tile([P, 512], FP, name="psv", tag="fp")
            for k in range(NK):
                nc.tensor.matmul(
                    ps,
                    cT[:, k, j * P:(j + 1) * P],
                    wv[k],
                    start=(k == 0),
                    stop=(k == NK - 1),
                )
            nc.vector.tensor_copy(
                V[j][:, :, 0:DH], ps.rearrange("p (h d) -> p h d", h=NH)
            )
            nc.vector.memset(V[j][:, :, DH:DH + 1], 1.0)

        # ---- minimal serial prefix ----
        for i in range(4):
            transpose_chunk(cT, c_d, i)
        wk = [wp.tile([P, CC], BF, name=f"wk{k}", tag="w") for k in range(NK)]
        wv = [wp.tile([P, CC], BF, name=f"wv{k}", tag="w") for k in range(NK)]
        wq = [wp.tile([P, CC], BF, name=f"wq{k}", tag="w") for k in range(NK)]
        for k in range(NK):
            nc.sync.dma_start(out=wk[k], in_=wk_d[k * P:(k + 1) * P, :])
            nc.sync.dma_start(out=wv[k], in_=wv_d[k * P:(k + 1) * P, :])
            nc.sync.dma_start(out=wq[k], in_=wq_d[k * P:(k + 1) * P, :])
        proj_chunk(KT, wk, cT, 0, 0)
        for j in range(4):
            v_chunk(j)
        for i in range(NIC):
            transpose_chunk(xT, x_d, i)
        proj_chunk(QT, wq, xT, 0, 0)
        proj_chunk(QT, wq, xT, 0, 1)

        # ---- j-granular filler: everything else streams through the
        # attention phase so the PE never drains (deadlines honored).
        def ct_u(i):
            return lambda: transpose_chunk(cT, c_d, i)

        def xt_u(i):
            return lambda: transpose_chunk(xT, x_d, i)

        def kt_u(m, i4):
            return lambda: proj_chunk(KT, wk, cT, m, i4)

        def qt_u(m, i4):
            return lambda: proj_chunk(QT, wq, xT, m, i4)

        def v_u(j):
            return lambda: v_chunk(j)

        filler = {
            (0, 0, 0): [ct_u(4), ct_u(5)],
            (0, 0, 1): [ct_u(6), ct_u(7)],
            (0, 0, 2): [kt_u(0, 1), v_u(4)],
            (0, 0, 3): [ct_u(8), v_u(5)],
            (0, 0, 4): [ct_u(9), v_u(6)],
            (0, 0, 5): [ct_u(10), v_u(7)],
            (0, 0, 6): [ct_u(11), kt_u(0, 2), v_u(8)],
            (0, 0, 7): [ct_u(12), v_u(9)],
            (0, 0, 8): [ct_u(13), v_u(10)],
            (0, 0, 9): [ct_u(14), v_u(11)],
            (0, 0, 10): [ct_u(15), kt_u(0, 3), v_u(12)],
            (0, 0, 11): [v_u(13)],
            (0, 0, 12): [v_u(14)],
            (0, 0, 13): [v_u(15)],
            (0, 1, 0): [kt_u(1, 0)], (0, 1, 2): [kt_u(1, 1)],
            (0, 1, 4): [kt_u(1, 2)], (0, 1, 6): [kt_u(1, 3)],
            (0, 1, 8): [qt_u(1, 0)], (0, 1, 11): [qt_u(1, 1)],
            (0, 2, 0): [kt_u(2, 0)], (0, 2, 4): [kt_u(2, 1)],
            (0, 2, 8): [kt_u(2, 2)], (0, 2, 12): [kt_u(2, 3)],
            (0, 3, 0): [qt_u(2, 0)], (0, 3, 8): [qt_u(2, 1)],
            (0, 4, 0): [kt_u(3, 0)], (0, 4, 4): [kt_u(3, 1)],
            (0, 4, 8): [kt_u(3, 2)], (0, 4, 12): [kt_u(3, 3)],
            (0, 5, 0): [qt_u(3, 0)], (0, 5, 8): [qt_u(3, 1)],
            (0, 6, 0): [xt_u(8)], (0, 6, 2): [xt_u(9)],
            (0, 6, 4): [xt_u(10)], (0, 6, 6): [xt_u(11)],
            (0, 6, 8): [xt_u(12)], (0, 6, 10): [xt_u(13)],
            (0, 6, 12): [xt_u(14)], (0, 6, 14): [xt_u(15)],
            (0, 7, 0): [qt_u(0, 2)], (0, 7, 8): [qt_u(0, 3)],
            (1, 0, 0): [qt_u(1, 2)], (1, 0, 8): [qt_u(1, 3)],
            (1, 2, 0): [qt_u(2, 2)], (1, 2, 8): [qt_u(2, 3)],
            (1, 4, 0): [qt_u(3, 2)], (1, 4, 8): [qt_u(3, 3)],
        }

        # ---------------- attention ----------------
        for imac in range(NIM):
            outs = [
                outp.tile([P, CC], FP, name=f"o{imac}_{b}", tag="o")
                for b in range(NIC)
            ]
            for h in range(NH):
                m = h // 2
                kt = KT[m]
                qt = QT[m]
                po = (h % 2) * DH
                at = apsum.tile([P, 2, 512], FP, name="at", tag="at")
                for j in range(NI):
                    for thunk in filler.get((imac, h, j), ()):
                        thunk()
                    sp = spsum.tile([P, IM], FP, name="sp", tag="sp")
                    for s in range(IM // 512):
                        nc.tensor.matmul(
                            sp[:, s * 512:(s + 1) * 512],
                            kt[po:po + DH, j * P:(j + 1) * P],
                            qt[po:po + DH,
                               imac * IM + s * 512:imac * IM + (s + 1) * 512],
                            start=True,
                            stop=True,
                        )
                    pt = ptp.tile([P, IM], BF, name="pt", tag="pt")
                    nc.scalar.activation(pt, sp, EXP, scale=SCALE)
                    for ic in range(NIC):
                        nc.tensor.matmul(
                            at[:, ic // 4, (ic % 4) * 65:(ic % 4) * 65 + 65],
                            pt[:, ic * P:(ic + 1) * P],
                            V[j][:, h, :],
                            start=(j == 0 and ic % 4 == 0),
                            stop=(j == NI - 1 and ic % 4 == 3),
                            skip_group_check=True,
                        )
                for ic in range(NIC):
                    blk = at[:, ic // 4, (ic % 4) * 65:(ic % 4) * 65 + 65]
                    rec = recp.tile([P, 1], FP, name="rec", tag="rec")
                    nc.vector.reciprocal(rec, blk[:, DH:DH + 1])
                    nc.vector.tensor_scalar_mul(
                        outs[ic][:, h * DH:(h + 1) * DH], blk[:, 0:DH], rec
                    )
            for blk in range(NIC):
                i0 = imac * IM + blk * P
                nc.sync.dma_start(out=out_d[i0:i0 + P, :], in_=outs[blk])


def _build():
    global _NC
    if _NC is not None:
        return _NC
    nc = bacc.Bacc(None, target_bir_lowering=False, debug=False)
    with TileContext(nc) as tc:
        with tc.tile_pool(name="dram", bufs=1, space="DRAM") as dram:
            x_d = dram.tile([SEQ, DIM], BF, kind="ExternalInput", name="x",
                            uniquify=False)
            c_d = dram.tile([SEQ, DIM], BF, kind="ExternalInput", name="ctx",
                            uniquify=False)
            wq_d = dram.tile([DIM, CC], BF, kind="ExternalInput", name="wq",
                             uniquify=False)
            wk_d = dram.tile([DIM, CC], BF, kind="ExternalInput", name="wk",
                             uniquify=False)
            wv_d = dram.tile([DIM, CC], BF, kind="ExternalInput", name="wv",
                             uniquify=False)
            out_d = dram.tile([SEQ, CC], FP, kind="ExternalOutput", name="out",
                              uniquify=False)
            _build_body(nc, tc, x_d, c_d, wq_d, wk_d, wv_d, out_d)
    nc.compile()
    _NC = nc
    return nc


def make_in_maps(x, context, Wq, Wkv):
    bf16 = ml_dtypes.bfloat16
    x = np.asarray(x, dtype=np.float32).astype(bf16)
    context = np.asarray(context, dtype=np.float32).astype(bf16)
    Wq = np.asarray(Wq, dtype=np.float32).astype(bf16)
    Wkv = np.asarray(Wkv, dtype=np.float32).astype(bf16)
    in_maps = []
    for core in range(8):
        b, hg = divmod(core, 2)
        c0 = hg * CC
        in_maps.append({
            "x": np.ascontiguousarray(x[b]),
            "ctx": np.ascontiguousarray(context[b]),
            "wq": np.ascontiguousarray(Wq[:, c0:c0 + CC]),
            "wk": np.ascontiguousarray(Wkv[:, c0:c0 + CC]),
            "wv": np.ascontiguousarray(Wkv[:, DIM + c0:DIM + c0 + CC]),
        })
    return in_maps


def run(x, context, Wq, Wkv, **run_kwargs):
    nc = _build()
    in_maps = make_in_maps(x, context, Wq, Wkv)
    res = run_bass_kernel_spmd(nc, in_maps, core_ids=list(range(8)), **run_kwargs)
    out = np.empty((4, SEQ, DIM), dtype=np.float32)
    for core in range(8):
        b, hg = divmod(core, 2)
        out[b, :, hg * CC:(hg + 1) * CC] = res.results[core]["out"]
    return out, res


def kernel(x, context, Wq, Wkv):
    out, _ = run(x, context, Wq, Wkv)
    return out
